# revision 11
# baseline (speedup 1.0000x reference)
"""Trainium2 Bass kernel for MultiInputModel (gnn_message_passing).

Math:
    gathered = state[:, idx]                       # [B, N, E]
    y   = tanh(einsum('bne,ne->bn', gathered, W) + b)   # [B, N]
    out = 500 * sigmoid(y @ Wf.T)                  # [B, A]

The gather + per-node linear is folded on the host into one dense matrix
A[c, n] = sum_e W[n, e] * [idx[n, e] == c], so the device computes two dense
matmuls with fused activations:
    yT  = tanh(A.T @ stateT + b)        # [N, Bc]  (node dim on partitions)
    out = 500 * sigmoid(yT.T @ WfT)     # [Bc, A]  (batch dim on partitions)

Sharding: batch 8192 -> 8 cores x 1024 rows; A / b / WfT replicated.
"""

import numpy as np

import concourse.bass as bass
import concourse.tile as tile
from concourse import bacc, mybir
from concourse.bass_utils import run_bass_kernel_spmd

N_CORES = 8
BATCH = 8192
B_CORE = BATCH // N_CORES  # 1024
STATE_DIM = 322
N_NODES = 256
ACTION = 4096

F32 = mybir.dt.float32
F32R = mybir.dt.float32r  # single-pass PE matmul (1 cyc/row vs 4 for fp32)
# contraction (state-dim) chunks: 322 = 128 + 128 + 66
C_CHUNKS = [(0, 128), (128, 128), (256, 66)]
AF = mybir.ActivationFunctionType


def _build_program() -> bass.Bass:
    # Bacc (not raw Bass): its compile pipeline splits multi-sem waits
    # (move_matmul_waits_to_ldweights / generate_event_semaphores) that the
    # TRN2 ISA requires — raw Bass programs fail walrus codegen on any
    # matmul with >1 semaphore wait.
    nc = bacc.Bacc("TRN2", target_bir_lowering=False, debug=False,
                   num_devices=N_CORES)

    stateT = nc.dram_tensor("stateT", [STATE_DIM, B_CORE], F32R, kind="ExternalInput")
    amat = nc.dram_tensor("amat", [STATE_DIM, N_NODES], F32R, kind="ExternalInput")
    bvec = nc.dram_tensor("bvec", [N_NODES, 1], F32, kind="ExternalInput")
    wfT = nc.dram_tensor("wfT", [N_NODES, ACTION], F32R, kind="ExternalInput")
    out = nc.dram_tensor("out", [B_CORE, ACTION], F32, kind="ExternalOutput")

    HB = 2048  # half-block: ACT/DVE/psum granularity (4 PSUM banks)

    with tile.TileContext(nc) as tc:
        with (
            tc.tile_pool(name="persist", bufs=1) as pp,
            tc.tile_pool(name="sig", bufs=3) as sigp,
            tc.tile_pool(name="obuf", bufs=3) as op,
            tc.tile_pool(name="ps", bufs=2, space="PSUM") as pso,
        ):
            # ---- load this core's stateT shard + A (phase A deps) first ----
            s_sb, a_sb = [], []
            for ci, (c0, cl) in enumerate(C_CHUNKS):
                st = pp.tile([128, B_CORE], F32R, tag=f"s{ci}")
                nc.sync.dma_start(out=st[:cl], in_=stateT[c0 : c0 + cl, :])
                s_sb.append(st)
                at = pp.tile([128, N_NODES], F32R, tag=f"a{ci}")
                nc.sync.dma_start(out=at[:cl], in_=amat[c0 : c0 + cl, :])
                a_sb.append(at)

            wf_sb, b_sb, y_sb = [], [], []
            for k in range(2):
                wt = pp.tile([128, ACTION], F32R, tag=f"wf{k}", name=f"wf{k}")
                wf_sb.append(wt)
                bt = pp.tile([128, 1], F32, tag=f"b{k}")
                nc.sync.dma_start(out=bt, in_=bvec[k * 128 : (k + 1) * 128, :])
                b_sb.append(bt)
                y_sb.append(pp.tile([128, B_CORE], F32R, tag=f"y{k}", name=f"y{k}"))
            # wf loads chunked by half-block (and interleaved k) so phase B's
            # first matmuls can start after ~2MB instead of the full 4MB.
            for h in range(ACTION // HB):
                for k in range(2):
                    nc.sync.dma_start(
                        out=wf_sb[k][:, h * HB : (h + 1) * HB],
                        in_=wfT[k * 128 : (k + 1) * 128, h * HB : (h + 1) * HB],
                    )

            # ---- phase A: yT = tanh(A.T @ stateT + b)  [256, B_CORE] ----
            ps = pso.tile([128, HB], F32, tag="ps", name="ps_a")
            for nk in range(2):
                for bj in range(B_CORE // 512):
                    dst = ps[:, nk * 1024 + bj * 512 : nk * 1024 + (bj + 1) * 512]
                    for ci, (c0, cl) in enumerate(C_CHUNKS):
                        nc.tensor.matmul(
                            dst,
                            lhsT=a_sb[ci][:cl, nk * 128 : (nk + 1) * 128],
                            rhs=s_sb[ci][:cl, bj * 512 : (bj + 1) * 512],
                            start=(ci == 0),
                            stop=(ci == len(C_CHUNKS) - 1),
                        )
            for nk in range(2):
                nc.scalar.activation(
                    out=y_sb[nk],
                    in_=ps[:, nk * 1024 : (nk + 1) * 1024],
                    func=AF.Tanh,
                    bias=b_sb[nk],
                    scale=1.0,
                )

            # ---- phase B: out = 500 * sigmoid(yT.T @ WfT)  [B_CORE, A] ----
            for bi in range(B_CORE // 128):
                ot = op.tile([128, ACTION], F32, tag="ot")
                for h in range(ACTION // HB):
                    ps = pso.tile([128, HB], F32, tag="ps")
                    for aj in range(HB // 512):
                        ai = h * (HB // 512) + aj
                        for k in range(2):
                            nc.tensor.matmul(
                                ps[:, aj * 512 : (aj + 1) * 512],
                                lhsT=y_sb[k][:, bi * 128 : (bi + 1) * 128],
                                rhs=wf_sb[k][:, ai * 512 : (ai + 1) * 512],
                                start=(k == 0),
                                stop=(k == 1),
                            )
                    sg = sigp.tile([128, HB], F32, tag="sg")
                    nc.scalar.activation(out=sg, in_=ps, func=AF.Sigmoid)
                    nc.vector.tensor_scalar_mul(
                        ot[:, h * HB : (h + 1) * HB], sg, 500.0
                    )
                nc.sync.dma_start(out=out[bi * 128 : (bi + 1) * 128, :], in_=ot)

    nc.finalize()  # Bacc.finalize -> compile(): reg alloc, wait splitting, ...
    return nc


def _prepare_in_maps(state, W, b, Wf, idx):
    state = np.asarray(state, dtype=np.float32)
    W = np.asarray(W, dtype=np.float32)
    b = np.asarray(b, dtype=np.float32)
    Wf = np.asarray(Wf, dtype=np.float32)
    idx = np.asarray(idx)

    # Fold gather+per-node-linear into one dense [STATE_DIM, N_NODES] matrix.
    amat = np.zeros((STATE_DIM, N_NODES), dtype=np.float32)
    cols = np.broadcast_to(np.arange(N_NODES, dtype=np.int64)[:, None], idx.shape)
    np.add.at(amat, (idx.astype(np.int64), cols), W)

    stateT = np.ascontiguousarray(state.T)  # [STATE_DIM, BATCH]
    wfT = np.ascontiguousarray(Wf.T)  # [N_NODES, ACTION]
    bcol = np.ascontiguousarray(b.reshape(N_NODES, 1))

    in_maps = []
    for i in range(N_CORES):
        in_maps.append(
            {
                "stateT": np.ascontiguousarray(
                    stateT[:, i * B_CORE : (i + 1) * B_CORE]
                ),
                "amat": amat,
                "bvec": bcol,
                "wfT": wfT,
            }
        )
    return in_maps


def _run(inputs: dict, trace: bool = False):
    nc = _build_program()
    in_maps = _prepare_in_maps(**inputs)
    res = run_bass_kernel_spmd(
        nc, in_maps, list(range(N_CORES)), trace=trace
    )
    out = np.concatenate([res.results[i]["out"] for i in range(N_CORES)], axis=0)
    return out, res


def kernel(**inputs) -> np.ndarray:
    out, _ = _run(inputs, trace=False)
    return out


if __name__ == "__main__":
    rng = np.random.default_rng(0)
    demo = {
        "state": rng.standard_normal((BATCH, STATE_DIM), dtype=np.float32),
        "W": rng.standard_normal((N_NODES, 27), dtype=np.float32),
        "b": rng.standard_normal(N_NODES, dtype=np.float32),
        "Wf": rng.standard_normal((ACTION, N_NODES), dtype=np.float32),
        "idx": rng.integers(0, STATE_DIM, size=(N_NODES, 27)).astype(np.int32),
    }
    o = kernel(**demo)
    print(o.shape, o.dtype)


# revision 12
# speedup vs baseline: 1.0784x; 1.0784x over previous
"""Trainium2 Bass kernel for MultiInputModel (gnn_message_passing).

Math:
    gathered = state[:, idx]                       # [B, N, E]
    y   = tanh(einsum('bne,ne->bn', gathered, W) + b)   # [B, N]
    out = 500 * sigmoid(y @ Wf.T)                  # [B, A]

The gather + per-node linear is folded on the host into one dense matrix
A[c, n] = sum_e W[n, e] * [idx[n, e] == c], so the device computes two dense
matmuls with fused activations:
    yT  = tanh(A.T @ stateT + b)        # [N, Bc]  (node dim on partitions)
    out = 500 * sigmoid(yT.T @ WfT)     # [Bc, A]  (batch dim on partitions)

Sharding: batch 8192 -> 8 cores x 1024 rows; A / b / WfT replicated.
"""

import numpy as np

import concourse.bass as bass
import concourse.tile as tile
from concourse import bacc, mybir
from concourse.bass_utils import run_bass_kernel_spmd

N_CORES = 8
BATCH = 8192
B_CORE = BATCH // N_CORES  # 1024
STATE_DIM = 322
N_NODES = 256
ACTION = 4096

F32 = mybir.dt.float32
F16 = mybir.dt.float16  # matmul operand dtype: 1 cyc/row, half the DMA bytes
# contraction (state-dim) chunks: 322 = 128 + 128 + 66
C_CHUNKS = [(0, 128), (128, 128), (256, 66)]
AF = mybir.ActivationFunctionType


def _build_program() -> bass.Bass:
    # Bacc (not raw Bass): its compile pipeline splits multi-sem waits
    # (move_matmul_waits_to_ldweights / generate_event_semaphores) that the
    # TRN2 ISA requires — raw Bass programs fail walrus codegen on any
    # matmul with >1 semaphore wait.
    nc = bacc.Bacc("TRN2", target_bir_lowering=False, debug=False,
                   num_devices=N_CORES)

    stateT = nc.dram_tensor("stateT", [STATE_DIM, B_CORE], F16, kind="ExternalInput")
    amat = nc.dram_tensor("amat", [STATE_DIM, N_NODES], F16, kind="ExternalInput")
    bvec = nc.dram_tensor("bvec", [N_NODES, 1], F32, kind="ExternalInput")
    wfT = nc.dram_tensor("wfT", [N_NODES, ACTION], F16, kind="ExternalInput")
    out = nc.dram_tensor("out", [B_CORE, ACTION], F32, kind="ExternalOutput")

    HB = 2048  # half-block: ACT/DVE/psum granularity (4 PSUM banks)

    with tile.TileContext(nc) as tc:
        with (
            tc.tile_pool(name="persist", bufs=1) as pp,
            tc.tile_pool(name="sig", bufs=3) as sigp,
            tc.tile_pool(name="obuf", bufs=3) as op,
            tc.tile_pool(name="ps", bufs=2, space="PSUM") as pso,
        ):
            # ---- load this core's stateT shard + A (phase A deps) first ----
            s_sb, a_sb = [], []
            for ci, (c0, cl) in enumerate(C_CHUNKS):
                st = pp.tile([128, B_CORE], F16, tag=f"s{ci}")
                nc.sync.dma_start(out=st[:cl], in_=stateT[c0 : c0 + cl, :])
                s_sb.append(st)
                at = pp.tile([128, N_NODES], F16, tag=f"a{ci}")
                nc.sync.dma_start(out=at[:cl], in_=amat[c0 : c0 + cl, :])
                a_sb.append(at)

            wf_sb, b_sb, y_sb = [], [], []
            for k in range(2):
                wt = pp.tile([128, ACTION], F16, tag=f"wf{k}", name=f"wf{k}")
                wf_sb.append(wt)
                bt = pp.tile([128, 1], F32, tag=f"b{k}")
                nc.sync.dma_start(out=bt, in_=bvec[k * 128 : (k + 1) * 128, :])
                b_sb.append(bt)
                y_sb.append(pp.tile([128, B_CORE], F16, tag=f"y{k}", name=f"y{k}"))
            # wf loads chunked by half-block (and interleaved k) so phase B's
            # first matmuls can start after ~2MB instead of the full 4MB.
            for h in range(ACTION // HB):
                for k in range(2):
                    nc.sync.dma_start(
                        out=wf_sb[k][:, h * HB : (h + 1) * HB],
                        in_=wfT[k * 128 : (k + 1) * 128, h * HB : (h + 1) * HB],
                    )

            # ---- phase A: yT = tanh(A.T @ stateT + b)  [256, B_CORE] ----
            ps = pso.tile([128, HB], F32, tag="ps", name="ps_a")
            for nk in range(2):
                for bj in range(B_CORE // 512):
                    dst = ps[:, nk * 1024 + bj * 512 : nk * 1024 + (bj + 1) * 512]
                    for ci, (c0, cl) in enumerate(C_CHUNKS):
                        nc.tensor.matmul(
                            dst,
                            lhsT=a_sb[ci][:cl, nk * 128 : (nk + 1) * 128],
                            rhs=s_sb[ci][:cl, bj * 512 : (bj + 1) * 512],
                            start=(ci == 0),
                            stop=(ci == len(C_CHUNKS) - 1),
                        )
            for nk in range(2):
                nc.scalar.activation(
                    out=y_sb[nk],
                    in_=ps[:, nk * 1024 : (nk + 1) * 1024],
                    func=AF.Tanh,
                    bias=b_sb[nk],
                    scale=1.0,
                )

            # ---- phase B: out = 500 * sigmoid(yT.T @ WfT)  [B_CORE, A] ----
            for bi in range(B_CORE // 128):
                ot = op.tile([128, ACTION], F32, tag="ot")
                for h in range(ACTION // HB):
                    ps = pso.tile([128, HB], F32, tag="ps")
                    for aj in range(HB // 512):
                        ai = h * (HB // 512) + aj
                        for k in range(2):
                            nc.tensor.matmul(
                                ps[:, aj * 512 : (aj + 1) * 512],
                                lhsT=y_sb[k][:, bi * 128 : (bi + 1) * 128],
                                rhs=wf_sb[k][:, ai * 512 : (ai + 1) * 512],
                                start=(k == 0),
                                stop=(k == 1),
                            )
                    sg = sigp.tile([128, HB], F32, tag="sg")
                    nc.scalar.activation(out=sg, in_=ps, func=AF.Sigmoid)
                    nc.vector.tensor_scalar_mul(
                        ot[:, h * HB : (h + 1) * HB], sg, 500.0
                    )
                nc.sync.dma_start(out=out[bi * 128 : (bi + 1) * 128, :], in_=ot)

    nc.finalize()  # Bacc.finalize -> compile(): reg alloc, wait splitting, ...
    return nc


def _prepare_in_maps(state, W, b, Wf, idx):
    state = np.asarray(state, dtype=np.float32)
    W = np.asarray(W, dtype=np.float32)
    b = np.asarray(b, dtype=np.float32)
    Wf = np.asarray(Wf, dtype=np.float32)
    idx = np.asarray(idx)

    # Fold gather+per-node-linear into one dense [STATE_DIM, N_NODES] matrix.
    amat = np.zeros((STATE_DIM, N_NODES), dtype=np.float32)
    cols = np.broadcast_to(np.arange(N_NODES, dtype=np.int64)[:, None], idx.shape)
    np.add.at(amat, (idx.astype(np.int64), cols), W)
    amat = amat.astype(np.float16)

    stateT = np.ascontiguousarray(state.T.astype(np.float16))  # [STATE_DIM, BATCH]
    wfT = np.ascontiguousarray(Wf.T.astype(np.float16))  # [N_NODES, ACTION]
    bcol = np.ascontiguousarray(b.reshape(N_NODES, 1))

    in_maps = []
    for i in range(N_CORES):
        in_maps.append(
            {
                "stateT": np.ascontiguousarray(
                    stateT[:, i * B_CORE : (i + 1) * B_CORE]
                ),
                "amat": amat,
                "bvec": bcol,
                "wfT": wfT,
            }
        )
    return in_maps


def _run(inputs: dict, trace: bool = False):
    nc = _build_program()
    in_maps = _prepare_in_maps(**inputs)
    res = run_bass_kernel_spmd(
        nc, in_maps, list(range(N_CORES)), trace=trace
    )
    out = np.concatenate([res.results[i]["out"] for i in range(N_CORES)], axis=0)
    return out, res


def kernel(**inputs) -> np.ndarray:
    out, _ = _run(inputs, trace=False)
    return out


if __name__ == "__main__":
    rng = np.random.default_rng(0)
    demo = {
        "state": rng.standard_normal((BATCH, STATE_DIM), dtype=np.float32),
        "W": rng.standard_normal((N_NODES, 27), dtype=np.float32),
        "b": rng.standard_normal(N_NODES, dtype=np.float32),
        "Wf": rng.standard_normal((ACTION, N_NODES), dtype=np.float32),
        "idx": rng.integers(0, STATE_DIM, size=(N_NODES, 27)).astype(np.int32),
    }
    o = kernel(**demo)
    print(o.shape, o.dtype)


# revision 13
# speedup vs baseline: 1.2333x; 1.1437x over previous
"""Trainium2 Bass kernel for MultiInputModel (gnn_message_passing).

Math:
    gathered = state[:, idx]                       # [B, N, E]
    y   = tanh(einsum('bne,ne->bn', gathered, W) + b)   # [B, N]
    out = 500 * sigmoid(y @ Wf.T)                  # [B, A]

The gather + per-node linear is folded on the host into one dense matrix
A[c, n] = sum_e W[n, e] * [idx[n, e] == c], so the device computes two dense
matmuls with fused activations:
    yT  = tanh(A.T @ stateT + b)        # [N, Bc]  (node dim on partitions)
    out = 500 * sigmoid(yT.T @ WfT)     # [Bc, A]  (batch dim on partitions)

Matmul operands are fp16 (1 PE cycle/row vs 4 for fp32; half the input DMA
bytes); accumulation is fp32 in PSUM and the activations/output stay fp32.

Sharding: batch 8192 -> 8 cores x 1024 rows; A / b / WfT replicated.

Input layout: everything the matmuls read is packed on the host into two
128-partition fp16 DRAM tensors so the whole input side is 3 large DMAs:
  pk1 [128, 3840]: stateT as 3 c-chunks [128,1024] + A as 3 c-chunks [128,256]
  pk2 [128, 8192]: WfT [256,4096] as [k0h0|k1h0|k0h1|k1h1] 2048-col groups
                   (h = 2048-wide half of the action dim), DMA'd per half.
"""

import numpy as np

import concourse.bass as bass
import concourse.tile as tile
from concourse import bacc, mybir
from concourse.bass_utils import run_bass_kernel_spmd

N_CORES = 8
BATCH = 8192
B_CORE = BATCH // N_CORES  # 1024
STATE_DIM = 322
N_NODES = 256
ACTION = 4096

F32 = mybir.dt.float32
F16 = mybir.dt.float16
# contraction (state-dim) chunks: 322 = 128 + 128 + 66
C_CHUNKS = [(0, 128), (128, 128), (256, 66)]
AF = mybir.ActivationFunctionType

HB = 2048  # half-block: ACT/DVE/psum granularity (4 PSUM banks)
PK1_COLS = 3 * B_CORE + 3 * N_NODES  # 3840
PK2_COLS = 2 * ACTION  # 8192


def _build_program() -> bass.Bass:
    # Bacc (not raw Bass): its compile pipeline splits multi-sem waits
    # (move_matmul_waits_to_ldweights / generate_event_semaphores) that the
    # TRN2 ISA requires — raw Bass programs fail walrus codegen on any
    # matmul with >1 semaphore wait.
    nc = bacc.Bacc("TRN2", target_bir_lowering=False, debug=False,
                   num_devices=N_CORES)

    pk1 = nc.dram_tensor("pk1", [128, PK1_COLS], F16, kind="ExternalInput")
    pk2 = nc.dram_tensor("pk2", [128, PK2_COLS], F16, kind="ExternalInput")
    bvec = nc.dram_tensor("bvec", [128, 2], F32, kind="ExternalInput")
    out = nc.dram_tensor("out", [B_CORE, ACTION], F32, kind="ExternalOutput")

    with tile.TileContext(nc) as tc:
        with (
            tc.tile_pool(name="persist", bufs=1) as pp,
            tc.tile_pool(name="sig", bufs=3) as sigp,
            tc.tile_pool(name="obuf", bufs=4) as op,
            tc.tile_pool(name="ps", bufs=2, space="PSUM") as pso,
        ):
            # ---- input DMAs: 1x phase-A pack, tiny bias, 2x wf halves ----
            t1 = pp.tile([128, PK1_COLS], F16, tag="t1")
            nc.sync.dma_start(out=t1, in_=pk1[:, :])
            bias_t = pp.tile([128, 2], F32, tag="bias")
            nc.sync.dma_start(out=bias_t, in_=bvec[:, :])
            t2 = pp.tile([128, PK2_COLS], F16, tag="t2")
            for h in range(2):
                nc.sync.dma_start(
                    out=t2[:, h * ACTION : (h + 1) * ACTION],
                    in_=pk2[:, h * ACTION : (h + 1) * ACTION],
                )

            def s_ap(ci, bsl):  # stateT chunk ci, batch slice
                return t1[:, ci * B_CORE : (ci + 1) * B_CORE][:, bsl]

            def a_ap(ci, nsl):  # A chunk ci, node slice
                base = 3 * B_CORE + ci * N_NODES
                return t1[:, base : base + N_NODES][:, nsl]

            def wf_ap(k, ai):  # WfT k-half, 512-wide action chunk ai
                h, aj = divmod(ai, HB // 512)
                base = h * ACTION + k * HB
                return t2[:, base + aj * 512 : base + (aj + 1) * 512]

            y_sb = [
                pp.tile([128, B_CORE], F16, tag=f"y{k}", name=f"y{k}")
                for k in range(2)
            ]

            # ---- phase A: yT = tanh(A.T @ stateT + b)  [256, B_CORE] ----
            ps = pso.tile([128, HB], F32, tag="ps", name="ps_a")
            for nk in range(2):
                for bj in range(B_CORE // 512):
                    dst = ps[:, nk * 1024 + bj * 512 : nk * 1024 + (bj + 1) * 512]
                    bsl = slice(bj * 512, (bj + 1) * 512)
                    nsl = slice(nk * 128, (nk + 1) * 128)
                    for ci, (c0, cl) in enumerate(C_CHUNKS):
                        nc.tensor.matmul(
                            dst,
                            lhsT=a_ap(ci, nsl)[:cl],
                            rhs=s_ap(ci, bsl)[:cl],
                            start=(ci == 0),
                            stop=(ci == len(C_CHUNKS) - 1),
                        )
            for nk in range(2):
                nc.scalar.activation(
                    out=y_sb[nk],
                    in_=ps[:, nk * 1024 : (nk + 1) * 1024],
                    func=AF.Tanh,
                    bias=bias_t[:, nk : nk + 1],
                    scale=1.0,
                )

            # ---- phase B: out = 500 * sigmoid(yT.T @ WfT)  [B_CORE, A] ----
            for bi in range(B_CORE // 128):
                ot = op.tile([128, ACTION], F32, tag="ot")
                for h in range(ACTION // HB):
                    ps = pso.tile([128, HB], F32, tag="ps")
                    for aj in range(HB // 512):
                        ai = h * (HB // 512) + aj
                        for k in range(2):
                            nc.tensor.matmul(
                                ps[:, aj * 512 : (aj + 1) * 512],
                                lhsT=y_sb[k][:, bi * 128 : (bi + 1) * 128],
                                rhs=wf_ap(k, ai),
                                start=(k == 0),
                                stop=(k == 1),
                            )
                    sg = sigp.tile([128, HB], F32, tag="sg")
                    nc.scalar.activation(out=sg, in_=ps, func=AF.Sigmoid)
                    nc.vector.tensor_scalar_mul(
                        ot[:, h * HB : (h + 1) * HB], sg, 500.0
                    )
                    nc.sync.dma_start(
                        out=out[bi * 128 : (bi + 1) * 128, h * HB : (h + 1) * HB],
                        in_=ot[:, h * HB : (h + 1) * HB],
                    )

    nc.finalize()  # Bacc.finalize -> compile(): reg alloc, wait splitting, ...
    return nc


def _prepare_in_maps(state, W, b, Wf, idx):
    state = np.asarray(state, dtype=np.float32)
    W = np.asarray(W, dtype=np.float32)
    b = np.asarray(b, dtype=np.float32)
    Wf = np.asarray(Wf, dtype=np.float32)
    idx = np.asarray(idx)

    # Fold gather+per-node-linear into one dense [STATE_DIM, N_NODES] matrix.
    amat = np.zeros((STATE_DIM, N_NODES), dtype=np.float32)
    cols = np.broadcast_to(np.arange(N_NODES, dtype=np.int64)[:, None], idx.shape)
    np.add.at(amat, (idx.astype(np.int64), cols), W)

    def to_chunks(m):  # [STATE_DIM, X] f32 -> [3, 128, X] f16 (zero padded)
        pad = np.zeros((3 * 128, m.shape[1]), dtype=np.float16)
        pad[:STATE_DIM] = m.astype(np.float16)
        return pad.reshape(3, 128, m.shape[1])

    a3 = to_chunks(amat)  # [3,128,256]
    wfT = np.ascontiguousarray(Wf.T.astype(np.float16))  # [256, 4096]
    # pk2 cols: [k0h0 | k1h0 | k0h1 | k1h1], each [128, 2048]
    pk2 = np.concatenate(
        [wfT[k * 128 : (k + 1) * 128, h * HB : (h + 1) * HB]
         for h in range(2) for k in range(2)],
        axis=1,
    )
    pk2 = np.ascontiguousarray(pk2)
    bias2 = np.ascontiguousarray(b.reshape(2, 128).T.astype(np.float32))  # [128,2]

    stateT = state.T.astype(np.float16)  # [STATE_DIM, BATCH]
    in_maps = []
    for i in range(N_CORES):
        s3 = to_chunks(stateT[:, i * B_CORE : (i + 1) * B_CORE])  # [3,128,1024]
        pk1 = np.concatenate(
            [s3[0], s3[1], s3[2], a3[0], a3[1], a3[2]], axis=1
        )  # [128, 3840]
        in_maps.append(
            {
                "pk1": np.ascontiguousarray(pk1),
                "pk2": pk2,
                "bvec": bias2,
            }
        )
    return in_maps


def _run(inputs: dict, trace: bool = False):
    nc = _build_program()
    in_maps = _prepare_in_maps(**inputs)
    res = run_bass_kernel_spmd(
        nc, in_maps, list(range(N_CORES)), trace=trace
    )
    out = np.concatenate([res.results[i]["out"] for i in range(N_CORES)], axis=0)
    return out, res


def kernel(**inputs) -> np.ndarray:
    out, _ = _run(inputs, trace=False)
    return out


if __name__ == "__main__":
    rng = np.random.default_rng(0)
    demo = {
        "state": rng.standard_normal((BATCH, STATE_DIM), dtype=np.float32),
        "W": rng.standard_normal((N_NODES, 27), dtype=np.float32),
        "b": rng.standard_normal(N_NODES, dtype=np.float32),
        "Wf": rng.standard_normal((ACTION, N_NODES), dtype=np.float32),
        "idx": rng.integers(0, STATE_DIM, size=(N_NODES, 27)).astype(np.int32),
    }
    o = kernel(**demo)
    print(o.shape, o.dtype)
